# revision 16
# baseline (speedup 1.0000x reference)
"""Multi-head attention kernel for 8 Trainium2 NeuronCores.

Strategy: tensor-parallel over heads. Core c owns heads (2c, 2c+1), i.e.
columns [128c, 128c+128) of the projection space.
  - column-parallel Wq/Wk/Wv: each core projects the full token stream onto
    its 128 columns; q/k are produced transposed ([cols, tok]) so the
    attention matmuls contract over the partition dim natively.
  - scores^T = k^T_blk.T @ q^T with softmax along the key dim (= partition),
    normalization deferred: E = exp(scale*scores + mask_bias), U^T = v.T @ E
    with an appended ones row giving sum(E) for free; ctx^T = U^T * (64/sum),
    and the final output is scaled by 1/64 (keeps the normalizer well inside
    fp16 range). The sum broadcast runs on the idle GPSIMD engine.
  - row-parallel Wo: each core emits a partial [4096, 1024] output; the host
    sums the 8 partials and adds bo.

The attention inner loop is paced by the Scalar engine's exp; projection and
output-projection matmuls for the *other* batch are interleaved into the
emission stream so the TensorEngine queue fills the exp-paced slack.

Matmul operands are fp16 (PE runs 1 cycle/row and FWL weight loads);
accumulation is fp32 in PSUM. Inputs are pre-transposed and cast to fp16 on
the host so all device-side DMA is contiguous and half-width.
"""

import numpy as np

import concourse.bass as bass
import concourse.tile as tile
from concourse import bacc, library_config, mybir
from concourse.bass_utils import run_bass_kernel_spmd

B, S, D, H = 2, 2048, 1024, 16
DH = D // H          # 64
NCORES = 8
HPC = H // NCORES    # heads per core = 2
CW = HPC * DH        # column width per core = 128
T = B * S            # 4096 tokens
SCALE = 1.0 / np.sqrt(DH)
RSCALE = 64.0        # normalizer pre-scale; undone at output projection

F32 = mybir.dt.float32
F16 = mybir.dt.float16

# v_s block layout: per 128-token block: [v_h0 (64) | ones | v_h1 (64) | ones]
VBLK = 2 * (DH + 1)  # 130

NKT = D // 128       # 8 contraction tiles for projections
NQC = S // 512       # 4 q-chunks per batch
NKB = S // 128       # 16 key blocks per batch
NTB = S // 128       # 16 token blocks per batch


def build_nc():
    nc = bacc.Bacc("TRN2", target_bir_lowering=False, debug=False,
                   num_devices=NCORES)

    qT_d = nc.declare_dram_parameter("qT", [D, T], F16, isOutput=False)
    kT_d = nc.declare_dram_parameter("kT", [D, T], F16, isOutput=False)
    vT_d = nc.declare_dram_parameter("vT", [D, T], F16, isOutput=False)
    wq_d = nc.declare_dram_parameter("wq", [D, CW], F16, isOutput=False)
    wk_d = nc.declare_dram_parameter("wk", [D, CW], F16, isOutput=False)
    wv_d = nc.declare_dram_parameter("wv", [D, CW], F16, isOutput=False)
    wo_d = nc.declare_dram_parameter("wo", [CW, D], F16, isOutput=False)
    bqkv_d = nc.declare_dram_parameter("bqkv", [CW, 3], F32, isOutput=False)
    maskb_d = nc.declare_dram_parameter("maskb", [128, B * NKB], F32,
                                        isOutput=False)
    ident_d = nc.declare_dram_parameter("ident", [128, 128], F16,
                                        isOutput=False)
    out_d = nc.declare_dram_parameter("out", [T, D], F32, isOutput=True)

    with tile.TileContext(nc) as tc:
        with (
            tc.tile_pool(name="weights", bufs=1) as wpool,
            tc.tile_pool(name="resident", bufs=1) as rpool,
            tc.tile_pool(name="proj_in", bufs=6) as inpool,
            tc.tile_pool(name="vt_tmp", bufs=2) as vtpool,
            tc.tile_pool(name="E", bufs=6) as epool,
            tc.tile_pool(name="r", bufs=4) as recpool,
            tc.tile_pool(name="Rsb", bufs=4) as rsbpool,
            tc.tile_pool(name="outsb", bufs=4) as outpool,
            # PSUM (8 banks): psA 2x[128,1024] = 4, psP 2x[128,512] = 2,
            # psU 2x[65,512] = 2
            tc.tile_pool(name="psA", bufs=2, space="PSUM") as psapool,
            tc.tile_pool(name="psP", bufs=2, space="PSUM") as psppool,
            tc.tile_pool(name="psU", bufs=2, space="PSUM") as psupool,
        ):
            nc.gpsimd.load_library(library_config.attn)

            # ---- load weights / constants (SBUF-resident) ----
            # w*_s[p, kt*CW + m] = w[kt*128 + p, m]
            wq_s = wpool.tile([128, NKT * CW], F16, tag="wq")
            wk_s = wpool.tile([128, NKT * CW], F16, tag="wk")
            wv_s = wpool.tile([128, NKT * CW], F16, tag="wv")
            for w_s, w_d in ((wq_s, wq_d), (wk_s, wk_d), (wv_s, wv_d)):
                nc.sync.dma_start(
                    w_s[:].rearrange("p (kt m) -> p kt m", m=CW),
                    w_d[:, :].rearrange("(kt p) m -> p kt m", p=128))
            wo_s = wpool.tile([128, D], F16, tag="wo")
            nc.sync.dma_start(wo_s[:], wo_d[:, :])
            bqkv_s = wpool.tile([CW, 3], F32, tag="bqkv")
            nc.sync.dma_start(bqkv_s[:], bqkv_d[:, :])
            maskb_s = wpool.tile([128, B * NKB], F32, tag="maskb")
            nc.sync.dma_start(maskb_s[:], maskb_d[:, :])
            ident_s = wpool.tile([128, 128], F16, tag="ident")
            nc.sync.dma_start(ident_s[:], ident_d[:, :])

            # ---- per-batch resident activation tiles ----
            qT_s = [rpool.tile([128, S], F16, tag=f"qT{b}", name=f"qT_s{b}")
                    for b in range(B)]
            kT_s = [rpool.tile([128, S], F16, tag=f"kT{b}", name=f"kT_s{b}")
                    for b in range(B)]
            v_s = [rpool.tile([128, NTB * VBLK], F16, tag=f"v{b}",
                              name=f"v_s{b}") for b in range(B)]
            ctxT_s = [rpool.tile([128, S], F16, tag=f"ctxT{b}",
                                 name=f"ctxT_s{b}") for b in range(B)]
            vt_tmp = [vtpool.tile([128, S], F16, tag="vt_tmp",
                                  name=f"vt_tmp{b}") for b in range(B)]

            for b in range(B):
                # ones columns interleaved into the v layout
                nc.vector.memset(
                    v_s[b][:].rearrange("p (k j) -> p k j", j=DH + 1)
                    [:, :, DH], 1.0)

            # ---- projection emitter: yields once per PE instruction ----
            def proj_gen(b):
                specs = ((kT_s[b], 1, wk_s, kT_d), (vt_tmp[b], 2, wv_s, vT_d),
                         (qT_s[b], 0, wq_s, qT_d))
                for dst_s, bias_col, w_s, src_d in specs:
                    for pp in range(2):
                        cols = slice(b * S + pp * 1024, b * S + (pp + 1) * 1024)
                        acc = [psppool.tile(
                                   [128, 512], F32, tag="psP",
                                   name=f"acc{b}_{bias_col}_{pp}_{j}")
                               for j in range(2)]
                        for kt in range(NKT):
                            src_t = inpool.tile(
                                [128, 1024], F16, tag="proj_in",
                                name=f"src{b}_{bias_col}_{pp}_{kt}")
                            nc.sync.dma_start(
                                src_t[:],
                                src_d[kt * 128:(kt + 1) * 128, cols])
                            for j in range(2):
                                nc.tensor.matmul(
                                    acc[j][:],
                                    w_s[:, kt * CW:(kt + 1) * CW],
                                    src_t[:, j * 512:(j + 1) * 512],
                                    start=(kt == 0), stop=(kt == NKT - 1))
                                yield
                        for j in range(2):
                            nc.vector.tensor_scalar_add(
                                dst_s[:, pp * 1024 + j * 512:
                                      pp * 1024 + (j + 1) * 512],
                                acc[j][:], bqkv_s[:, bias_col:bias_col + 1])
                    if bias_col == 2:
                        # PE-transpose v^T into normal layout (with ones gaps)
                        for t in range(NTB):
                            pst = psapool.tile([128, 128], F16, tag="psA",
                                               name=f"pst{b}_{t}")
                            nc.tensor.transpose(
                                pst[:], vt_tmp[b][:, t * 128:(t + 1) * 128],
                                ident_s[:])
                            yield
                            nc.vector.tensor_copy(
                                v_s[b][:, t * VBLK:t * VBLK + DH],
                                pst[:, 0:DH])
                            nc.vector.tensor_copy(
                                v_s[b][:, t * VBLK + DH + 1:
                                       t * VBLK + 2 * DH + 1],
                                pst[:, DH:2 * DH])

            # ---- output-projection emitter for one q-chunk of a batch ----
            def outproj_gen(b, qc):
                for t in range(qc * NTB // NQC, (qc + 1) * NTB // NQC):
                    for ch in range(2):
                        acc = psppool.tile([128, 512], F32, tag="psP",
                                           name=f"psO{b}_{t}_{ch}")
                        nc.tensor.matmul(
                            acc[:],
                            ctxT_s[b][:, t * 128:(t + 1) * 128],
                            wo_s[:, ch * 512:(ch + 1) * 512],
                            start=True, stop=True)
                        yield
                        o_sb = outpool.tile([128, 512], F32, tag="outsb",
                                            name=f"o_sb{b}_{t}_{ch}")
                        nc.vector.tensor_scalar_mul(o_sb[:], acc[:],
                                                    1.0 / RSCALE)
                        nc.sync.dma_start(
                            out_d[b * S + t * 128:b * S + (t + 1) * 128,
                                  ch * 512:(ch + 1) * 512],
                            o_sb[:])

            # background PE work queue, driven from the attention loop
            bg = []

            def drive(n):
                for _ in range(n):
                    while bg:
                        try:
                            next(bg[0])
                            break
                        except StopIteration:
                            bg.pop(0)
                    else:
                        return

            def drain():
                while bg:
                    for _ in bg.pop(0):
                        pass

            # ---- attention for one batch, driving background work ----
            def attention(b):
                for qc in range(NQC):
                    qsl = slice(qc * 512, (qc + 1) * 512)
                    psU = [psupool.tile([DH + 1, 512], F32, tag="psU",
                                        name=f"psU{b}_{qc}_{h}")
                           for h in range(HPC)]
                    pend = []  # deferred U-matmul emissions (2-kb pipeline)
                    for kb in range(NKB):
                        psE = psapool.tile([128, 1024], F32, tag="psA",
                                           name=f"psE{b}_{qc}_{kb}")
                        for h in range(HPC):
                            rows = slice(64 * h, 64 * h + 64)
                            nc.tensor.matmul(
                                psE[:, h * 512:(h + 1) * 512],
                                kT_s[b][rows, kb * 128:(kb + 1) * 128],
                                qT_s[b][rows, qsl],
                                start=True, stop=True)
                        e_sb = epool.tile([128, 1024], F16, tag="E",
                                          name=f"e{b}_{qc}_{kb}")
                        nc.scalar.activation(
                            e_sb[:], psE[:],
                            mybir.ActivationFunctionType.Exp,
                            bias=maskb_s[:, b * NKB + kb:b * NKB + kb + 1],
                            scale=SCALE)
                        drive(2)
                        pend.append((e_sb, kb))
                        if len(pend) > 2:
                            emit_u(b, psU, *pend.pop(0))
                    for args in pend:
                        emit_u(b, psU, *args)

                    # normalizer: broadcast sums on GPSIMD (per-head tiles;
                    # non-zero base partitions miscompute on HW), then a
                    # full-width reciprocal on DVE
                    for h in range(HPC):
                        s_sb = recpool.tile([1, 512], F32, tag="r",
                                            name=f"s_sb{b}_{qc}_{h}")
                        nc.vector.tensor_scalar_mul(
                            s_sb[:], psU[h][DH:DH + 1, :], 1.0 / RSCALE)
                        R_sb = rsbpool.tile([DH, 512], F32, tag="Rsb",
                                            name=f"R_sb{b}_{qc}_{h}")
                        nc.gpsimd.partition_broadcast(R_sb[:], s_sb[:])
                        Rrec = rsbpool.tile([DH, 512], F32, tag="Rrec",
                                            name=f"Rrec{b}_{qc}_{h}")
                        with nc.allow_low_precision(
                                reason="softmax normalizer, fp32"):
                            nc.vector.reciprocal(Rrec[:], R_sb[:])
                        rows = slice(64 * h, 64 * h + 64)
                        nc.vector.tensor_mul(
                            ctxT_s[b][rows, qsl], psU[h][0:DH, :],
                            Rrec[:, :])

            def emit_u(b, psU, e_sb, kb):
                for h in range(HPC):
                    nc.tensor.matmul(
                        psU[h][:],
                        v_s[b][:, kb * VBLK + h * (DH + 1):
                               kb * VBLK + (h + 1) * (DH + 1)],
                        e_sb[:, h * 512:(h + 1) * 512],
                        start=(kb == 0), stop=(kb == NKB - 1))

            # ---- phase schedule ----
            for _ in proj_gen(0):          # phase A: proj b0 (DMA-paced)
                pass
            bg.append(proj_gen(1))         # phase B: attn b0 + proj b1
            attention(0)
            bg.append(outproj_gen(0, 0))   # phase C: attn b1 + outproj b0
            for qc in range(1, NQC):
                bg.append(outproj_gen(0, qc))
            attention(1)
            for qc in range(NQC):          # b1 outproj emitted at tail; the
                bg.append(outproj_gen(1, qc))  # scheduler overlaps via deps
            drain()

    nc.compile()
    return nc


_NC_CACHE = []
LAST_RESULT = {}


def kernel(**inputs):
    query = np.asarray(inputs["query"], np.float32)
    key = np.asarray(inputs["key"], np.float32)
    value = np.asarray(inputs["value"], np.float32)
    mask = np.asarray(inputs["mask"], np.float32)
    Wq = np.asarray(inputs["Wq"], np.float32)
    Wk = np.asarray(inputs["Wk"], np.float32)
    Wv = np.asarray(inputs["Wv"], np.float32)
    Wo = np.asarray(inputs["Wo"], np.float32)
    bq = np.asarray(inputs["bq"], np.float32)
    bk = np.asarray(inputs["bk"], np.float32)
    bv = np.asarray(inputs["bv"], np.float32)
    bo = np.asarray(inputs["bo"], np.float32)

    qT = np.ascontiguousarray(query.reshape(T, D).T.astype(np.float16))
    kT = np.ascontiguousarray(key.reshape(T, D).T.astype(np.float16))
    vT = np.ascontiguousarray(value.reshape(T, D).T.astype(np.float16))
    # maskb[p, b*16+kb] = -1e9 * mask[b, 0, 0, kb*128+p]
    maskb = np.ascontiguousarray(
        (mask[:, 0, 0, :] * np.float32(-1e9))
        .reshape(B, S // 128, 128).transpose(2, 0, 1).reshape(128, -1))
    ident = np.eye(128, dtype=np.float16)

    in_maps = []
    for c in range(NCORES):
        cols = slice(CW * c, CW * (c + 1))
        in_maps.append({
            "qT": qT, "kT": kT, "vT": vT,
            "wq": np.ascontiguousarray(Wq[:, cols].astype(np.float16)),
            "wk": np.ascontiguousarray(Wk[:, cols].astype(np.float16)),
            "wv": np.ascontiguousarray(Wv[:, cols].astype(np.float16)),
            "wo": np.ascontiguousarray(Wo[cols, :].astype(np.float16)),
            "bqkv": np.ascontiguousarray(
                np.stack([bq[cols], bk[cols], bv[cols]], axis=1)),
            "maskb": maskb,
            "ident": ident,
        })

    if not _NC_CACHE:
        _NC_CACHE.append(build_nc())
    nc = _NC_CACHE[0]

    import os
    trace = bool(os.environ.get("KERNEL_TRACE"))
    res = run_bass_kernel_spmd(nc, in_maps, core_ids=list(range(NCORES)),
                               trace=trace)
    LAST_RESULT["res"] = res
    out = np.zeros((T, D), np.float64)
    for c in range(NCORES):
        out += res.results[c]["out"].astype(np.float64)
    out = (out + bo.astype(np.float64)).astype(np.float32)
    return out.reshape(B, S, D)
